# revision 17
# baseline (speedup 1.0000x reference)
"""Trainium2 Bass kernel for nn_LowFreqPenaltyLoss.

Computes mean(|einsum('ih,nchw,jw->ncij', Ch, delta, Cw)|) for
delta [256, 3, 256, 256] f32, Ch/Cw the 8x256 unnormalized DCT-II bases.

Strategy (data-parallel over batch, 8 cores):
  - each core gets 32 batches = 96 images [256, 256] (24 MiB), loaded as 12
    groups of 8 images via FLAT SWDGE DMAs: partition p takes a 16 KiB
    contiguous HBM chunk (image q = p//16, rows 16*(p%16)..+15), cast
    f32->bf16 inline. Flat descriptors (vs the old per-row 1 KiB gather)
    cut per-packet overhead: the old layout streamed at ~275 GB/s read-side;
    HBM-per-NC cap is ~358 GB/s.
  - stage A (contract h): h = 16*(p%16)+r is split across partition and
    free axes, so the DCT-H contraction uses 16 accumulating matmuls with
    block-diagonal weights wA8[p, r, 8*(p//16)+i] = Ch[i, 16*(p%16)+r],
    psumA[8q+i, w] over 64 partitions.
  - stage B (contract w): ACT copies psumA -> SBUF (casts bf16), 2 PE
    transposes of [64,128] chunks (each into its own PSUM bank), DVE copies
    out, then matmul with CwT -> ps2[j, (q,i)], fused |.|+sum on DVE into a
    per-partition accumulator acc[8,1].
  - final: 8-partition matmul reduction scaled by 1/49152; host sums the 8
    per-core partials. bf16 inputs + f32 PSUM accumulation give ~2e-4
    relative error on the final scalar.
"""

import sys
import types

for _p in ("/root/.axon_site/_ro/trn_rl_repo", "/opt/trn_rl_repo"):
    if _p not in sys.path:
        sys.path.append(_p)

import numpy as np
from contextlib import ExitStack

import concourse.bass as bass
import concourse.tile as tile
from concourse import mybir, bass_utils
from concourse._compat import with_exitstack
from concourse.vector_clock import ScopedClock

# ---------------------------------------------------------------------------
# Workarounds for this image.
# ---------------------------------------------------------------------------

# walrus on this image rejects >1 sync-wait on one CTRL instruction; split the
# Tile exit-drain's waits across follow-up nops (same engine, program order).
# Also: the stock tail (barrier + per-sem clear + barrier) costs ~8-10us of
# EVSEM butterfly at kernel end. The kernel is one-shot per NEFF execution and
# NRT re-initialises semaphores per execution, so keep only the drain + DMA
# completion waits.
_ORIG_DAB = tile.TileContext._drain_and_barrier
_USE_STOCK_TAIL = False


def _patched_drain_and_barrier(self, tick_clock, wait_clock):
    if _USE_STOCK_TAIL:
        return _ORIG_DAB(self, tick_clock, wait_clock)
    nc = self.nc
    drain_inst = nc.sync.drain()
    wait_clock.add_sem_waits(
        drain_inst.ins, ScopedClock({None: tick_clock.global_clock})
    )
    si = drain_inst.ins.sync_info
    waits = list(si.on_wait) if si and si.on_wait else []
    if len(waits) > 1:
        drain_inst.ins.sync_info = mybir.SyncInfo(
            on_wait=[waits[0]], on_update=list(si.on_update or [])
        )
        for w in waits[1:]:
            nop = nc.sync.nop(nofuse=True, hint="drain_wait_split")
            nop.ins.sync_info = mybir.SyncInfo(on_wait=[w], on_update=[])
    popped = nc._tile_sem_poison_stack.pop()
    assert popped is self._sem_poison


tile.TileContext._drain_and_barrier = _patched_drain_and_barrier

# zero-egress container: profiling artifact upload must stay local.
bass_utils.upload_artifacts = lambda d: d


def _strip_main_barrier(nc):
    """Drop the prologue all-engine barrier AND the dead const memsets in
    'main': the barrier's only role is to fence the framework preamble (dead
    const memsets + per-engine table loads) from the kernel, but per-engine
    program order already covers the table loads, and nothing reads the
    const tiles (verified: no instruction references const-* memrefs).  Each
    engine then branches into the kernel as soon as its own init finishes
    instead of waiting for the slowest engine (~1.4us of startup)."""
    for fn in nc.m.functions:
        for bb in fn.blocks:
            if bb.name != "main":
                continue
            bb.instructions[:] = [
                i for i in bb.instructions
                if not isinstance(
                    i,
                    (mybir.InstEventSemaphore, mybir.InstDrain, mybir.InstMemset),
                )
            ]


def _split_multi_waits(nc):
    """walrus on this image rejects >1 sync-wait per instruction: hoist extra
    waits onto fresh NoOps inserted just before, on the same engine."""
    for fn in nc.m.functions:
        for bb in fn.blocks:
            new_insts = []
            for inst in bb.instructions:
                si = inst.sync_info
                waits = list(si.on_wait) if si and si.on_wait else []
                if len(waits) > 1:
                    for w in waits[:-1]:
                        nop = mybir.InstNoOp(
                            name=nc.get_next_instruction_name(),
                            sync_info=mybir.SyncInfo(on_wait=[w], on_update=[]),
                            bass_nofuse=True,
                            engine=inst.engine,
                        )
                        new_insts.append(nop)
                    inst.sync_info = mybir.SyncInfo(
                        on_wait=[waits[-1]], on_update=list(si.on_update or [])
                    )
                new_insts.append(inst)
            bb.instructions[:] = new_insts

# ---------------------------------------------------------------------------
# Problem constants (hardcoded; kernel.py must be self-contained).
# ---------------------------------------------------------------------------

B, C, H, W = 256, 3, 256, 256
LOW_A = LOW_B = 8
N_CORES = 8
IMGS_PER_CORE = (B // N_CORES) * C          # 96
GRP = 8                                     # images per group (2 MiB f32)
N_GROUPS = IMGS_PER_CORE // GRP             # 12
ROWS = H // (128 // GRP)                    # 16 rows per partition
TOTAL_LOW = B * C * LOW_A * LOW_B           # 49152 -> mean divisor

F32 = mybir.dt.float32
BF16 = mybir.dt.bfloat16


def _dct_basis(K, N):
    n = np.arange(N, dtype=np.float64)
    k = np.arange(K, dtype=np.float64)
    return (2.0 * np.cos(np.pi * (2.0 * n[None, :] + 1.0) * k[:, None] / (2.0 * N))).astype(
        np.float32
    )


def _make_consts():
    Ch = _dct_basis(LOW_A, H)   # [8, 256]
    Cw = _dct_basis(LOW_B, W)   # [8, 256]
    # Block-diagonal DCT-H weights for the flat layout: partition p holds
    # image q = p//16, rows h = 16*(p%16) + r.  wa8[p, r, 8q+i] = Ch[i, h].
    wa8 = np.zeros((128, ROWS, 64), np.float32)
    for p in range(128):
        q, pp = p // 16, p % 16
        for r in range(ROWS):
            wa8[p, r, 8 * q:8 * q + 8] = Ch[:, ROWS * pp + r]
    # 4-image tail groups: image q = p//32, rows h = 8*(p%32) + r.
    wa4 = np.zeros((128, 8, 32), np.float32)
    for p in range(128):
        q, pp = p // 32, p % 32
        for r in range(8):
            wa4[p, r, 8 * q:8 * q + 8] = Ch[:, 8 * pp + r]
    # cwt[p, wc, j] = Cw[j, wc*128+p]
    cwt = np.zeros((128, 2, LOW_B), np.float32)
    for wc in range(2):
        cwt[:, wc, :] = Cw[:, wc * 128:(wc + 1) * 128].T
    import ml_dtypes
    bf16 = ml_dtypes.bfloat16
    ident = np.eye(128, dtype=bf16)
    # f32 weights for the HWDGE-loaded rows (14, 15) of each 8-img group
    wa8f = np.ascontiguousarray(wa8[:, 14:16, :])
    return wa8.astype(bf16), wa8f, wa4.astype(bf16), cwt.astype(bf16), ident


WA8, WA8F, WA4, CWT, IDENT = _make_consts()


# ---------------------------------------------------------------------------
# Kernel body (per core; SPMD over 8 cores).
# ---------------------------------------------------------------------------

@with_exitstack
def _lowfreq_kernel(ctx: ExitStack, tc, out_ap, delta_ap, wa8_ap, wa8f_ap,
                    wa4_ap, cwt_ap, ident_ap):
    nc = tc.nc

    const_pool = ctx.enter_context(tc.tile_pool(name="const", bufs=1))
    in8_pool = ctx.enter_context(tc.tile_pool(name="in8", bufs=N_GROUPS - 1))
    inf_pool = ctx.enter_context(tc.tile_pool(name="inf", bufs=N_GROUPS - 1))
    in4_pool = ctx.enter_context(tc.tile_pool(name="in4", bufs=2))
    sS_pool = ctx.enter_context(tc.tile_pool(name="sS", bufs=3))
    tS_pool = ctx.enter_context(tc.tile_pool(name="tS", bufs=3))
    red_pool = ctx.enter_context(tc.tile_pool(name="red", bufs=2))
    acc_pool = ctx.enter_context(tc.tile_pool(name="acc", bufs=1))
    psA_pool = ctx.enter_context(tc.tile_pool(name="psA", bufs=3, space="PSUM"))
    psT_pool = ctx.enter_context(tc.tile_pool(name="psT", bufs=3, space="PSUM"))
    ps2_pool = ctx.enter_context(tc.tile_pool(name="ps2", bufs=2, space="PSUM"))

    # constants (HWDGE queue; lands well before first compute)
    wa8 = const_pool.tile([128, ROWS, 64], BF16)
    nc.sync.dma_start(wa8[:], wa8_ap)
    wa8f = const_pool.tile([128, 2, 64], F32)
    nc.sync.dma_start(wa8f[:], wa8f_ap)
    wa4 = const_pool.tile([128, 8, 32], BF16)
    nc.sync.dma_start(wa4[:], wa4_ap)
    cwt = const_pool.tile([128, 2, LOW_B], BF16)
    nc.sync.dma_start(cwt[:], cwt_ap)
    ident = const_pool.tile([128, 128], BF16)
    nc.sync.dma_start(ident[:], ident_ap)

    acc = acc_pool.tile([8, 1], F32)
    nc.vector.memset(acc[:], 0.0)

    # issue ALL input DMAs upfront: partition p <- 16 KiB contiguous HBM
    # (image q = p//16, rows 16*(p%16)..+15), SWDGE casts f32->bf16 inline.
    # Mid-stream groups are PAIRED into one 4 MiB DMA each (fewer exit-drain
    # sem waits); the tail is 1x8 + 2x4 images so the post-stream critical
    # path is short.  Each entry: (tile, sub-index or None, n_img).
    subs = []
    for g in range(N_GROUPS - 1):
        gt = in8_pool.tile([128, ROWS - 2, 256], BF16, tag="gt8")
        gf = inf_pool.tile([128, 2, 256], F32, tag="gf8")
        src = delta_ap[GRP * g:GRP * g + GRP]
        rr = src.rearrange("q (pp r) w -> (q pp) r w", pp=16, r=ROWS)
        # rows 0..13 via SWDGE (casts bf16); rows 14..15 via HWDGE as raw
        # f32: HWDGE has no SBUF descriptor ring, so these bytes are immune
        # to the SDMA-engine-15 slow mode that intermittently gates the
        # stream (engine 15 then carries 14/16 of its nominal load).
        nc.gpsimd.dma_start(gt[:], rr[:, 0:ROWS - 2, :])
        nc.sync.dma_start(gf[:], rr[:, ROWS - 2:ROWS, :])
        subs.append(((gt, gf), None, GRP))
    for t in range(2):
        gt4 = in4_pool.tile([128, 8, 256], BF16, tag="gt4")
        src = delta_ap[88 + 4 * t:92 + 4 * t]
        nc.gpsimd.dma_start(
            gt4[:],
            src.rearrange("q (pp r) w -> (q pp) (r w)", pp=32, r=8),
        )
        subs.append((gt4, None, 4))

    def stage_a(sub):
        gt, g2, n_img = sub
        rows = H * n_img // 128
        n_out = 8 * n_img
        psumA = psA_pool.tile([n_out, 256], F32, tag="psA")
        if isinstance(gt, tuple):
            gtb, gtf = gt
            for r in range(rows - 2):
                nc.tensor.matmul(
                    psumA[:], lhsT=wa8[:, r, :], rhs=gtb[:, r, :],
                    start=(r == 0), stop=False,
                )
            for rr in range(2):
                nc.tensor.matmul(
                    psumA[:], lhsT=wa8f[:, rr, :], rhs=gtf[:, rr, :],
                    start=False, stop=(rr == 1),
                )
        else:
            wA = wa8 if n_img == GRP else wa4
            for r in range(rows):
                nc.tensor.matmul(
                    psumA[:],
                    lhsT=wA[:, r, :],
                    rhs=gt[:, r, :],
                    start=(r == 0),
                    stop=(r == rows - 1),
                )
        # PSUM -> SBUF with f32->bf16 cast (ACT engine; off the PE timeline)
        sA = sS_pool.tile([n_out, 256], BF16, tag="sA")
        nc.scalar.copy(sA[:], psumA[:])
        return sA, n_out

    def stage_b(sA, n_out):
        # 2 PE transposes (own PSUM tiles: transpose-mode output must start
        # at a bank boundary on HW), DVE copies out, then contract w into
        # ps2[j, (q,i)], fused |.|+sum, accumulate.
        tS = tS_pool.tile([128, 2, n_out], BF16, tag="tS")
        for wc in range(2):
            tp = psT_pool.tile([128, n_out], BF16, tag="tp")
            nc.tensor.transpose(
                tp[:],
                sA[:, 128 * wc:128 * wc + 128],
                ident[0:n_out, 0:n_out],
            )
            nc.vector.tensor_copy(tS[:, wc, :], tp[:])
        ps2 = ps2_pool.tile([8, n_out], F32, tag="ps2")
        for wc in range(2):
            nc.tensor.matmul(
                ps2[:],
                lhsT=cwt[:, wc, :],
                rhs=tS[:, wc, :],
                start=(wc == 0),
                stop=(wc == 1),
            )
        red = red_pool.tile([8, 1], F32)
        nc.vector.tensor_reduce(
            red[:], ps2[:], axis=mybir.AxisListType.X,
            op=mybir.AluOpType.add, apply_absolute_value=True,
        )
        nc.vector.tensor_add(acc[:], acc[:], red[:])

    # Software pipeline: emit stage B of group g-1 AFTER stage A of group g,
    # so the PE (which executes in program order) never stalls mid-stream on
    # the ACT/DVE round-trips of stage B.
    prev = None
    for sub in subs:
        cur = stage_a(sub)
        if prev is not None:
            stage_b(*prev)
        prev = cur
    stage_b(*prev)

    # ship the 8 per-partition partials; the host does the final sum + mean
    # (shaves the PE matmul + DVE copy off the post-stream critical path).
    # Issue on Scalar (HWDGE) so it overlaps the SP exit-drain sem walk.
    nc.scalar.dma_start(out_ap, acc[:])


# ---------------------------------------------------------------------------
# Build + run.
# ---------------------------------------------------------------------------

_CACHED_NC = None


def _build(for_sim=False):
    global _CACHED_NC, _USE_STOCK_TAIL
    if not for_sim and _CACHED_NC is not None:
        return _CACHED_NC
    _USE_STOCK_TAIL = for_sim
    nc = bass.Bass("TRN2", target_bir_lowering=False, debug=False)
    delta = nc.dram_tensor("delta", [IMGS_PER_CORE, H, W], F32, kind="ExternalInput")
    wa8 = nc.dram_tensor("wa8", list(WA8.shape), BF16, kind="ExternalInput")
    wa8f = nc.dram_tensor("wa8f", list(WA8F.shape), F32, kind="ExternalInput")
    wa4 = nc.dram_tensor("wa4", list(WA4.shape), BF16, kind="ExternalInput")
    cwt = nc.dram_tensor("cwt", list(CWT.shape), BF16, kind="ExternalInput")
    ident = nc.dram_tensor("ident", list(IDENT.shape), BF16, kind="ExternalInput")
    out = nc.dram_tensor("out", [8, 1], F32, kind="ExternalOutput")

    with tile.TileContext(nc) as tc:
        _lowfreq_kernel(
            tc, out.ap(), delta.ap(), wa8.ap(), wa8f.ap(), wa4.ap(), cwt.ap(),
            ident.ap()
        )
    _USE_STOCK_TAIL = False
    if for_sim:
        return nc
    _strip_main_barrier(nc)
    _split_multi_waits(nc)
    _CACHED_NC = nc
    return nc


def _run(delta, **spmd_kwargs):
    import os
    os.environ["JAX_PLATFORMS"] = "axon"   # harness may have pinned cpu for the reference
    nc = _build()
    delta = np.ascontiguousarray(np.asarray(delta, dtype=np.float32))
    assert delta.shape == (B, C, H, W)
    shards = delta.reshape(N_CORES, IMGS_PER_CORE, H, W)
    in_maps = [
        {
            "delta": shards[i],
            "wa8": WA8,
            "wa8f": WA8F,
            "wa4": WA4,
            "cwt": CWT,
            "ident": IDENT,
        }
        for i in range(N_CORES)
    ]
    try:
        res = bass_utils.run_bass_kernel_spmd(
            nc, in_maps, core_ids=list(range(N_CORES)), **spmd_kwargs
        )
    except Exception:
        # transient NRT_EXEC_UNIT_UNRECOVERABLE has been observed on this
        # terminal; one retry typically succeeds.
        res = bass_utils.run_bass_kernel_spmd(
            nc, in_maps, core_ids=list(range(N_CORES)), **spmd_kwargs
        )
    total = np.float64(0.0)
    for r in res.results:
        total += np.asarray(r["out"], np.float64).sum()
    return np.float32(total / TOTAL_LOW).reshape(()), res


def kernel(delta):
    out, _ = _run(delta)
    return out
